# revision 24
# baseline (speedup 1.0000x reference)
"""DonutSwin window self-attention on 8 Trainium2 NeuronCores.

Strategy (data-parallel over windows, 512 windows/core, contiguous-98
window-pair layout, software-pipelined blocks):
- Host: shard hidden_states over cores, pre-transpose each shard to
  xT [512, 25088] (feature-major), fold 1/sqrt(hd) into Wq, precompute a
  masked exp(rel-pos-bias) table ebm[k 98, oc, hq, 1, q 98] whose
  off-diagonal 49-blocks are exactly zero (kills cross-window terms).
- Device per core, per 8-window block (4 window-pairs of 98 tokens):
  * qT/kT = W^T @ xT feature-major (quad layout: partitions = 4 heads x 32)
  * v = x @ Wv token-major with M=98 (both windows of a pair in one
    stationary matmul), v_sb [98, 16, 32+ones-col]
  * scores^T[k, q]: one 2-bank PSUM tile per (quad, hq-pair); each bank
    only ever takes matmuls from its single row strip (32*hq, 0) -- mixing
    row strips within a bank faults the PE.  Stationary kT is padded to
    M=128 token-columns so fast-weight-load (FWL) engages; the extra
    output partitions hold finite next-pair scores and are never read.
  * e = exp(scores) on ACT (one op per 2-bank tile), e *= ebm on DVE
    (applies rel-pos bias AND zeroes cross-window blocks exactly)
  * ctx[q, (h, d|sum)] = e^T @ v_aug: one K=98 matmul per (pair, head),
    e padded to 128 columns for FWL; 8 heads per PSUM bank [128, 8, 33]
  * normalize: DVE reciprocal of the ones-column sums + DVE multiply
    straight from PSUM, DMA out token-major
  * emission order per block: scores rounds first, then next block's
    DMA + qkv chains, ctx rounds last -- keeps independent work behind
    any stalled dependent chain in the in-order PE queue.
- Output gathered to [4096, 49, 512] fp32.
"""

import numpy as np

WIN = 7
DIM = 512
HEADS = 16
HD = DIM // HEADS  # 32
B = 4096
N = WIN * WIN  # 49
N2 = 2 * N  # 98
NCORES = 8
BC = B // NCORES  # 512 windows per core
T = BC * N  # 25088 tokens per core
OCT = 8  # windows per block
NOCT = BC // OCT  # 64
TOK_OCT = OCT * N  # 392
PAIRS = OCT // 2  # 4 window-pairs per block

_NC_CACHE = {}
CFG = {
    "qkcopy": "scalar",  # engine for qT/kT PSUM->SBUF copies
    "vcopy": "vector",
    "n_eb_pool": 0,  # how many of the 16 ebmul ops go to gpsimd
    "ps_qk": 2, "ps_v": 1, "ps_s": 2, "ps_c": 2,
    "score_m128": True,  # pad scores stationary to M=128 (FWL)
    "e_pad128": True,  # pad e_sb cols to 128 (FWL for ctx matmuls)
}


def _build_nc(noct=NOCT):
    import concourse.bass as bass  # noqa: F401
    import concourse.tile as tile
    from concourse import bacc, mybir

    f32 = mybir.dt.float32
    f16 = mybir.dt.float16
    AFT = mybir.ActivationFunctionType

    nc = bacc.Bacc("TRN2", target_bir_lowering=False, debug=False)

    T_ = noct * TOK_OCT
    xT_d = nc.dram_tensor("xt", [DIM, T_], f16, kind="ExternalInput")
    wq_d = nc.dram_tensor("wq", [DIM, DIM], f16, kind="ExternalInput")
    wk_d = nc.dram_tensor("wk", [DIM, DIM], f16, kind="ExternalInput")
    wv_d = nc.dram_tensor("wv", [DIM, DIM], f16, kind="ExternalInput")
    # masked exp(bias)^T: [k 98, oc 4, hq 4, q 98]
    ebm_d = nc.dram_tensor("ebm", [N2, 4, 4, 1, N2], f16, kind="ExternalInput")
    out_d = nc.dram_tensor("out", [T_, DIM], f16, kind="ExternalOutput")

    EC = 128 if CFG["e_pad128"] else N2  # e_sb column count

    with tile.TileContext(nc) as tc:
        with (
            tc.tile_pool(name="consts", bufs=1) as consts,
            tc.tile_pool(name="xt", bufs=3) as xt_pool,
            tc.tile_pool(name="qk", bufs=3) as qk_pool,
            tc.tile_pool(name="vsb", bufs=9) as v_pool,
            tc.tile_pool(name="esb", bufs=20) as e_pool,
            tc.tile_pool(name="osb", bufs=9) as o_pool,
            tc.tile_pool(name="small", bufs=16) as small,
            tc.tile_pool(name="ps_qk", bufs=CFG["ps_qk"], space="PSUM") as ps_qk,
            tc.tile_pool(name="ps_v", bufs=CFG["ps_v"], space="PSUM") as ps_v,
            tc.tile_pool(name="ps_s", bufs=CFG["ps_s"], space="PSUM") as ps_s,
            tc.tile_pool(name="ps_c", bufs=CFG["ps_c"], space="PSUM") as ps_c,
        ):
            wq_sb = consts.tile([128, 4, DIM], f16, tag="wq")
            wk_sb = consts.tile([128, 4, DIM], f16, tag="wk")
            wv_sb = consts.tile([128, 4, DIM], f16, tag="wv")
            for w_sb, w_d in ((wq_sb, wq_d), (wk_sb, wk_d), (wv_sb, wv_d)):
                nc.sync.dma_start(
                    out=w_sb,
                    in_=w_d[:, :].rearrange("(i p) o -> p i o", p=128),
                )
            ebm_sb = consts.tile([N2, 4, 4, 1, N2], f16, tag="ebm")
            nc.sync.dma_start(out=ebm_sb, in_=ebm_d[:, :, :, :, :])

            qk_engine = getattr(nc, CFG["qkcopy"])

            def make_block(oct_i):
                t0 = oct_i * TOK_OCT
                st = {}

                def dma():
                    xt = xt_pool.tile([128, 4, TOK_OCT], f16, tag="xt", name="xt")
                    nc.sync.dma_start(
                        out=xt,
                        in_=xT_d[:, t0 : t0 + TOK_OCT].rearrange(
                            "(i p) t -> p i t", p=128
                        ),
                    )
                    st["xt"] = xt

                def qk_chain(dst, w_sb, oc):
                    ps = ps_qk.tile([128, DIM], f32, tag="ps_qv", name="ps")
                    for ic in range(4):
                        nc.tensor.matmul(
                            ps[:, 0:TOK_OCT],
                            w_sb[:, ic, oc * 128 : (oc + 1) * 128],
                            st["xt"][:, ic, :],
                            start=(ic == 0),
                            stop=(ic == 3),
                        )
                    if CFG["qkcopy"] == "scalar":
                        nc.scalar.activation(
                            out=dst[:, oc, :], in_=ps[:, 0:TOK_OCT], func=AFT.Copy
                        )
                    else:
                        qk_engine.tensor_copy(dst[:, oc, :], ps[:, 0:TOK_OCT])

                def v_chain(pr):
                    c0 = pr * N2
                    ps = ps_qk.tile([128, DIM], f32, tag="ps_qv", name="ps")
                    for ic in range(4):
                        nc.tensor.matmul(
                            ps[0:N2, :],
                            st["xt"][:, ic, c0 : c0 + N2],
                            wv_sb[:, ic, :],
                            start=(ic == 0),
                            stop=(ic == 3),
                        )
                    v_sb = v_pool.tile([N2, HEADS, HD + 1], f16, tag="vsb", name="v_sb")
                    if CFG["vcopy"] == "scalar":
                        nc.scalar.activation(
                            out=v_sb[:, :, 0:HD],
                            in_=ps[0:N2, :].rearrange("p (h d) -> p h d", d=HD),
                            func=AFT.Copy,
                        )
                    else:
                        nc.vector.tensor_copy(
                            v_sb[:, :, 0:HD],
                            ps[0:N2, :].rearrange("p (h d) -> p h d", d=HD),
                        )
                    nc.vector.memset(v_sb[:, :, HD : HD + 1], 1.0)
                    st["v"][pr] = v_sb

                def qkv():
                    qT = qk_pool.tile([128, 4, TOK_OCT], f16, tag="qT", name="qT")
                    kT = qk_pool.tile([128, 4, TOK_OCT], f16, tag="kT", name="kT")
                    st["qT"], st["kT"] = qT, kT
                    st["v"] = [None] * PAIRS
                    qk_chain(qT, wq_sb, 0)
                    qk_chain(kT, wk_sb, 0)
                    qk_chain(qT, wq_sb, 1)
                    qk_chain(kT, wk_sb, 1)
                    v_chain(0)
                    qk_chain(qT, wq_sb, 2)
                    v_chain(1)
                    qk_chain(kT, wk_sb, 2)
                    v_chain(2)
                    qk_chain(qT, wq_sb, 3)
                    v_chain(3)
                    qk_chain(kT, wk_sb, 3)
                    st["e"] = [None] * HEADS
                    st["out"] = [
                        o_pool.tile([N2, HEADS, HD], f16, tag="osb", name="out_sb")
                        for _ in range(PAIRS)
                    ]

                def scores_round(oc):
                    # one 2-bank PSUM tile per (oc, hq-pair); each bank only
                    # ever takes matmuls from a single row strip (32*hq, 0).
                    qT, kT = st["qT"], st["kT"]
                    for hqp in range(2):
                        s_ps = ps_s.tile([128, 2, 512], f32, tag="ps_s", name="s_ps")
                        sv = s_ps[:, :, 0 : 4 * N2].rearrange(
                            "p b (r q) -> p b r q", q=N2
                        )
                        for pr in range(PAIRS):
                            c0 = pr * N2
                            km = (
                                128
                                if (CFG["score_m128"] and c0 + 128 <= TOK_OCT)
                                else N2
                            )
                            for hh in range(2):
                                hq = 2 * hqp + hh
                                nc.tensor.matmul(
                                    sv[0:km, hh, pr, :],
                                    kT[32 * hq : 32 * hq + 32, oc, c0 : c0 + km],
                                    qT[32 * hq : 32 * hq + 32, oc, c0 : c0 + N2],
                                    start=True,
                                    stop=True,
                                    tile_position=(32 * hq, 0),
                                )
                        e_sb = e_pool.tile([N2, 2, 4, EC], f16, tag="esb", name="e_sb")
                        nc.scalar.activation(
                            out=e_sb[:, :, :, 0:N2],
                            in_=sv[0:N2, :, :, :],
                            func=AFT.Exp,
                        )
                        nc.vector.tensor_mul(
                            e_sb[:, :, :, 0:N2],
                            e_sb[:, :, :, 0:N2],
                            ebm_sb[:, oc, 2 * hqp : 2 * hqp + 2, :, :].to_broadcast(
                                [N2, 2, 4, N2]
                            ),
                        )
                        st["e"][oc * 2 + hqp] = e_sb

                def ctx_round(half):
                    for pr in range(PAIRS):
                        v_sb = st["v"][pr]
                        out_sb = st["out"][pr]
                        c_ps = ps_c.tile([128, 8, HD + 1], f32, tag="ps_c", name="c_ps")
                        for hh in range(8):
                            h = half * 8 + hh
                            oc, hq = divmod(h, 4)
                            e_sb = st["e"][oc * 2 + hq // 2]
                            nc.tensor.matmul(
                                c_ps[0:EC, hh, :],
                                e_sb[:, hq % 2, pr, :],
                                v_sb[:, h, :],
                                start=True,
                                stop=True,
                                tile_position=(0, 0),
                            )
                        rec = small.tile([N2, 8, 1], f32, tag="rec", name="rec")
                        nc.vector.reciprocal(
                            rec[:, :, :], c_ps[0:N2, :, HD : HD + 1]
                        )
                        nc.vector.tensor_mul(
                            out_sb[:, half * 8 : half * 8 + 8, :],
                            c_ps[0:N2, :, 0:HD],
                            rec[:, :, :].to_broadcast([N2, 8, HD]),
                        )
                        if half == 1:
                            r0 = t0 + pr * N2
                            nc.sync.dma_start(
                                out=out_d[r0 : r0 + N2, :],
                                in_=out_sb[:, :, :].rearrange("p h d -> p (h d)"),
                            )

                st["dma"] = dma
                st["qkv"] = qkv
                st["scores_round"] = scores_round
                st["ctx_round"] = ctx_round
                return st

            # software pipeline: block i+1's projections are emitted between
            # scores(i) and the final ctx round of block i so the in-order PE
            # queue always has independent work during exp/ebmul waits.
            cur = make_block(0)
            cur["dma"]()
            cur["qkv"]()
            for oct_i in range(noct):
                nxt = make_block(oct_i + 1) if oct_i + 1 < noct else None
                if nxt is not None:
                    nxt["dma"]()
                cur["scores_round"](0)
                cur["scores_round"](1)
                cur["scores_round"](2)
                cur["scores_round"](3)
                if nxt is not None:
                    nxt["qkv"]()
                cur["ctx_round"](0)
                cur["ctx_round"](1)
                cur = nxt

    nc.compile()
    return nc


def _host_prep(hidden_states, Wq, bq, Wk, bk, Wv, bv, rel_pos_bias_table, rel_pos_index):
    scale = float(HD) ** -0.5
    x = np.ascontiguousarray(np.asarray(hidden_states, dtype=np.float32)).reshape(
        B * N, DIM
    )
    wq = np.ascontiguousarray(
        (np.asarray(Wq, dtype=np.float32) * scale).astype(np.float16)
    )
    wk = np.ascontiguousarray(np.asarray(Wk, dtype=np.float16))
    wv = np.ascontiguousarray(np.asarray(Wv, dtype=np.float16))
    bq_ = np.asarray(bq, dtype=np.float32)
    bk_ = np.asarray(bk, dtype=np.float32)
    bv_ = np.asarray(bv, dtype=np.float32)
    assert (
        np.abs(bq_).max() == 0 and np.abs(bk_).max() == 0 and np.abs(bv_).max() == 0
    ), "nonzero qkv bias unsupported in v2 kernel"

    table = np.asarray(rel_pos_bias_table, dtype=np.float32)
    idx = np.asarray(rel_pos_index, dtype=np.int64)
    bias = table[idx.reshape(-1)].reshape(N, N, HEADS)  # [q, k, h]
    biasT = np.exp(bias.transpose(2, 1, 0))  # exp, [h, k, q]
    # masked two-window table: ebm[k 98, oc, hq, q 98]
    ebm = np.zeros((N2, 4, 4, 1, N2), dtype=np.float16)
    for h in range(HEADS):
        oc, hq = divmod(h, 4)
        ebm[0:N, oc, hq, 0, 0:N] = biasT[h]
        ebm[N:N2, oc, hq, 0, N:N2] = biasT[h]

    in_maps = []
    for c in range(NCORES):
        xc = x[c * T : (c + 1) * T]  # [T, DIM]
        xT = np.ascontiguousarray(xc.T.astype(np.float16))  # [DIM, T]
        in_maps.append({"xt": xT, "wq": wq, "wk": wk, "wv": wv, "ebm": ebm})
    return in_maps


def kernel(hidden_states, Wq, bq, Wk, bk, Wv, bv, rel_pos_bias_table, rel_pos_index):
    from concourse.bass_utils import run_bass_kernel_spmd

    in_maps = _host_prep(
        hidden_states, Wq, bq, Wk, bk, Wv, bv, rel_pos_bias_table, rel_pos_index
    )
    if "nc" not in _NC_CACHE:
        _NC_CACHE["nc"] = _build_nc()
    nc = _NC_CACHE["nc"]

    res = run_bass_kernel_spmd(nc, in_maps, core_ids=list(range(NCORES)))
    out = np.empty((B * N, DIM), dtype=np.float32)
    for c in range(NCORES):
        out[c * T : (c + 1) * T] = res.results[c]["out"]
    return out.reshape(B, N, DIM)


# revision 25
# speedup vs baseline: 2.3280x; 2.3280x over previous
"""DonutSwin window self-attention on 8 Trainium2 NeuronCores.

Strategy (data-parallel over windows, 512 windows/core, contiguous-98
window-pair layout, software-pipelined blocks):
- Host: shard hidden_states over cores, pre-transpose each shard to
  xT [512, 25088] (feature-major), fold 1/sqrt(hd) into Wq, precompute a
  masked exp(rel-pos-bias) table ebm[k 98, oc, hq, 1, q 98] whose
  off-diagonal 49-blocks are exactly zero (kills cross-window terms).
- Device per core, per 8-window block (4 window-pairs of 98 tokens):
  * qT/kT = W^T @ xT feature-major (quad layout: partitions = 4 heads x 32)
  * v = x @ Wv token-major with M=98 (both windows of a pair in one
    stationary matmul), v_sb [98, 16, 32+ones-col]
  * scores^T[k, q]: one 2-bank PSUM tile per (quad, hq-pair); each bank
    only ever takes matmuls from its single row strip (32*hq, 0) -- mixing
    row strips within a bank faults the PE.  Stationary kT is padded to
    M=128 token-columns so fast-weight-load (FWL) engages; the extra
    output partitions hold finite next-pair scores and are never read.
  * e = exp(scores) on ACT (one op per 2-bank tile), e *= ebm on DVE
    (applies rel-pos bias AND zeroes cross-window blocks exactly)
  * ctx[q, (h, d|sum)] = e^T @ v_aug: one K=98 matmul per (pair, head),
    e padded to 128 columns for FWL; 8 heads per PSUM bank [128, 8, 33]
  * normalize: DVE reciprocal of the ones-column sums + DVE multiply
    straight from PSUM, DMA out token-major
  * emission order per block: scores rounds first, then next block's
    DMA + qkv chains, ctx rounds last -- keeps independent work behind
    any stalled dependent chain in the in-order PE queue.
- Output gathered to [4096, 49, 512] fp32.
"""

import numpy as np

WIN = 7
DIM = 512
HEADS = 16
HD = DIM // HEADS  # 32
B = 4096
N = WIN * WIN  # 49
N2 = 2 * N  # 98
NCORES = 8
BC = B // NCORES  # 512 windows per core
T = BC * N  # 25088 tokens per core
# blocks of 5 window-pairs (490 tokens, fills PSUM banks) with a few
# 4-pair blocks to cover 512 windows: 48*5 + 4*4 = 256 pairs
PMAX = 5
TOKMAX = PMAX * N2  # 490

_NC_CACHE = {}
CFG = {
    "qkcopy": "scalar",  # engine for qT/kT PSUM->SBUF copies
    "vcopy": "vector",
    "n_eb_pool": 0,  # how many of the 16 ebmul ops go to gpsimd
    "ps_qk": 2, "ps_v": 1, "ps_s": 2, "ps_c": 2,
    "score_m128": True,  # pad scores stationary to M=128 (FWL)
    "e_pad128": True,  # pad e_sb cols to 128 (FWL for ctx matmuls)
}


def _block_plan(nwin):
    npairs_total = nwin // 2
    n5, rem = divmod(npairs_total, PMAX)
    plan = [PMAX] * n5
    while rem >= 4:
        plan.append(4)
        rem -= 4
    if rem:
        n4 = 0
        while rem and plan and plan[-1] == 4:
            plan.pop()
            rem += 4
        while rem >= 4:
            plan.append(4)
            rem -= 4
        if rem:
            plan.append(rem)
    return plan


def _build_nc(nwin=BC):
    import concourse.bass as bass  # noqa: F401
    import concourse.tile as tile
    from concourse import bacc, mybir

    f32 = mybir.dt.float32
    f16 = mybir.dt.float16
    AFT = mybir.ActivationFunctionType

    nc = bacc.Bacc("TRN2", target_bir_lowering=False, debug=False)

    plan = _block_plan(nwin)
    T_ = (nwin // 2) * N2
    xT_d = nc.dram_tensor("xt", [DIM, T_], f16, kind="ExternalInput")
    wq_d = nc.dram_tensor("wq", [DIM, DIM], f16, kind="ExternalInput")
    wk_d = nc.dram_tensor("wk", [DIM, DIM], f16, kind="ExternalInput")
    wv_d = nc.dram_tensor("wv", [DIM, DIM], f16, kind="ExternalInput")
    # masked exp(bias)^T: [k 98, oc 4, hq 4, q 98]
    ebm_d = nc.dram_tensor("ebm", [N2, 4, 4, 1, N2], f16, kind="ExternalInput")
    out_d = nc.dram_tensor("out", [T_, DIM], f16, kind="ExternalOutput")

    EC = 128 if CFG["e_pad128"] else N2  # e_sb column count

    with tile.TileContext(nc) as tc:
        with (
            tc.tile_pool(name="consts", bufs=1) as consts,
            tc.tile_pool(name="xt", bufs=3) as xt_pool,
            tc.tile_pool(name="qk", bufs=3) as qk_pool,
            tc.tile_pool(name="vsb", bufs=9) as v_pool,
            tc.tile_pool(name="esb", bufs=20) as e_pool,
            tc.tile_pool(name="osb", bufs=9) as o_pool,
            tc.tile_pool(name="small", bufs=16) as small,
            tc.tile_pool(name="ps_qk", bufs=CFG["ps_qk"], space="PSUM") as ps_qk,
            tc.tile_pool(name="ps_v", bufs=CFG["ps_v"], space="PSUM") as ps_v,
            tc.tile_pool(name="ps_s", bufs=CFG["ps_s"], space="PSUM") as ps_s,
            tc.tile_pool(name="ps_c", bufs=CFG["ps_c"], space="PSUM") as ps_c,
        ):
            wq_sb = consts.tile([128, 4, DIM], f16, tag="wq")
            wk_sb = consts.tile([128, 4, DIM], f16, tag="wk")
            wv_sb = consts.tile([128, 4, DIM], f16, tag="wv")
            for w_sb, w_d in ((wq_sb, wq_d), (wk_sb, wk_d), (wv_sb, wv_d)):
                nc.sync.dma_start(
                    out=w_sb,
                    in_=w_d[:, :].rearrange("(i p) o -> p i o", p=128),
                )
            ebm_sb = consts.tile([N2, 4, 4, 1, N2], f16, tag="ebm")
            nc.sync.dma_start(out=ebm_sb, in_=ebm_d[:, :, :, :, :])

            qk_engine = getattr(nc, CFG["qkcopy"])

            def make_block(t0, npairs):
                TOK = npairs * N2
                st = {}

                def dma():
                    xt = xt_pool.tile([128, 4, TOKMAX], f16, tag="xt", name="xt")
                    nc.sync.dma_start(
                        out=xt[:, :, 0:TOK],
                        in_=xT_d[:, t0 : t0 + TOK].rearrange(
                            "(i p) t -> p i t", p=128
                        ),
                    )
                    st["xt"] = xt

                def qk_chain(dst, w_sb, oc):
                    ps = ps_qk.tile([128, DIM], f32, tag="ps_qv", name="ps")
                    for ic in range(4):
                        nc.tensor.matmul(
                            ps[:, 0:TOK],
                            w_sb[:, ic, oc * 128 : (oc + 1) * 128],
                            st["xt"][:, ic, 0:TOK],
                            start=(ic == 0),
                            stop=(ic == 3),
                        )
                    if CFG["qkcopy"] == "scalar":
                        nc.scalar.activation(
                            out=dst[:, oc, 0:TOK], in_=ps[:, 0:TOK], func=AFT.Copy
                        )
                    else:
                        qk_engine.tensor_copy(dst[:, oc, 0:TOK], ps[:, 0:TOK])

                def v_chain(pr):
                    c0 = pr * N2
                    ps = ps_qk.tile([128, DIM], f32, tag="ps_qv", name="ps")
                    for ic in range(4):
                        nc.tensor.matmul(
                            ps[0:N2, :],
                            st["xt"][:, ic, c0 : c0 + N2],
                            wv_sb[:, ic, :],
                            start=(ic == 0),
                            stop=(ic == 3),
                        )
                    v_sb = v_pool.tile([N2, HEADS, HD + 1], f16, tag="vsb", name="v_sb")
                    if CFG["vcopy"] == "scalar":
                        nc.scalar.activation(
                            out=v_sb[:, :, 0:HD],
                            in_=ps[0:N2, :].rearrange("p (h d) -> p h d", d=HD),
                            func=AFT.Copy,
                        )
                    else:
                        nc.vector.tensor_copy(
                            v_sb[:, :, 0:HD],
                            ps[0:N2, :].rearrange("p (h d) -> p h d", d=HD),
                        )
                    nc.vector.memset(v_sb[:, :, HD : HD + 1], 1.0)
                    st["v"][pr] = v_sb

                def qkv():
                    qT = qk_pool.tile([128, 4, TOKMAX], f16, tag="qT", name="qT")
                    kT = qk_pool.tile([128, 4, TOKMAX], f16, tag="kT", name="kT")
                    st["qT"], st["kT"] = qT, kT
                    st["v"] = [None] * npairs
                    order = [
                        ("q", 0), ("k", 0), ("q", 1), ("k", 1), ("v", 0),
                        ("q", 2), ("v", 1), ("k", 2), ("v", 2), ("q", 3),
                        ("v", 3), ("k", 3), ("v", 4),
                    ]
                    for kind, idx in order:
                        if kind == "q":
                            qk_chain(qT, wq_sb, idx)
                        elif kind == "k":
                            qk_chain(kT, wk_sb, idx)
                        elif idx < npairs:
                            v_chain(idx)
                    st["e"] = [None] * HEADS
                    st["out"] = [
                        o_pool.tile([N2, HEADS, HD], f16, tag="osb", name="out_sb")
                        for _ in range(npairs)
                    ]

                def scores_round(oc):
                    # one 2-bank PSUM tile per (oc, hq-pair); each bank only
                    # ever takes matmuls from a single row strip (32*hq, 0).
                    qT, kT = st["qT"], st["kT"]
                    for hqp in range(2):
                        s_ps = ps_s.tile([128, 2, 512], f32, tag="ps_s", name="s_ps")
                        sv = s_ps[:, :, 0 : npairs * N2].rearrange(
                            "p b (r q) -> p b r q", q=N2
                        )
                        for pr in range(npairs):
                            c0 = pr * N2
                            km = (
                                128
                                if (CFG["score_m128"] and c0 + 128 <= TOK)
                                else N2
                            )
                            for hh in range(2):
                                hq = 2 * hqp + hh
                                nc.tensor.matmul(
                                    sv[0:km, hh, pr, :],
                                    kT[32 * hq : 32 * hq + 32, oc, c0 : c0 + km],
                                    qT[32 * hq : 32 * hq + 32, oc, c0 : c0 + N2],
                                    start=True,
                                    stop=True,
                                    tile_position=(32 * hq, 0),
                                )
                        e_sb = e_pool.tile(
                            [N2, 2, PMAX, EC], f16, tag="esb", name="e_sb"
                        )
                        nc.scalar.activation(
                            out=e_sb[:, :, 0:npairs, 0:N2],
                            in_=sv[0:N2, :, :, :],
                            func=AFT.Exp,
                        )
                        nc.vector.tensor_mul(
                            e_sb[:, :, 0:npairs, 0:N2],
                            e_sb[:, :, 0:npairs, 0:N2],
                            ebm_sb[:, oc, 2 * hqp : 2 * hqp + 2, :, :].to_broadcast(
                                [N2, 2, npairs, N2]
                            ),
                        )
                        st["e"][oc * 2 + hqp] = e_sb

                def ctx_round(half):
                    for pr in range(npairs):
                        v_sb = st["v"][pr]
                        out_sb = st["out"][pr]
                        c_ps = ps_c.tile([128, 8, HD + 1], f32, tag="ps_c", name="c_ps")
                        for hh in range(8):
                            h = half * 8 + hh
                            oc, hq = divmod(h, 4)
                            e_sb = st["e"][oc * 2 + hq // 2]
                            nc.tensor.matmul(
                                c_ps[0:EC, hh, :],
                                e_sb[:, hq % 2, pr, :],
                                v_sb[:, h, :],
                                start=True,
                                stop=True,
                                tile_position=(0, 0),
                            )
                        rec = small.tile([N2, 8, 1], f32, tag="rec", name="rec")
                        nc.vector.reciprocal(
                            rec[:, :, :], c_ps[0:N2, :, HD : HD + 1]
                        )
                        nc.vector.tensor_mul(
                            out_sb[:, half * 8 : half * 8 + 8, :],
                            c_ps[0:N2, :, 0:HD],
                            rec[:, :, :].to_broadcast([N2, 8, HD]),
                        )
                        if half == 1:
                            r0 = t0 + pr * N2
                            nc.sync.dma_start(
                                out=out_d[r0 : r0 + N2, :],
                                in_=out_sb[:, :, :].rearrange("p h d -> p (h d)"),
                            )

                st["dma"] = dma
                st["qkv"] = qkv
                st["scores_round"] = scores_round
                st["ctx_round"] = ctx_round
                return st

            # software pipeline: block i+1's projections are emitted between
            # scores(i) and the final ctx round of block i so the in-order PE
            # queue always has independent work during exp/ebmul waits.
            starts = []
            t_acc = 0
            for npairs in plan:
                starts.append((t_acc, npairs))
                t_acc += npairs * N2
            cur = make_block(*starts[0])
            cur["dma"]()
            cur["qkv"]()
            for bi in range(len(starts)):
                nxt = make_block(*starts[bi + 1]) if bi + 1 < len(starts) else None
                if nxt is not None:
                    nxt["dma"]()
                cur["scores_round"](0)
                cur["scores_round"](1)
                cur["scores_round"](2)
                cur["scores_round"](3)
                if nxt is not None:
                    nxt["qkv"]()
                cur["ctx_round"](0)
                cur["ctx_round"](1)
                cur = nxt

    nc.compile()
    return nc


def _host_prep(hidden_states, Wq, bq, Wk, bk, Wv, bv, rel_pos_bias_table, rel_pos_index):
    scale = float(HD) ** -0.5
    x = np.ascontiguousarray(np.asarray(hidden_states, dtype=np.float32)).reshape(
        B * N, DIM
    )
    wq = np.ascontiguousarray(
        (np.asarray(Wq, dtype=np.float32) * scale).astype(np.float16)
    )
    wk = np.ascontiguousarray(np.asarray(Wk, dtype=np.float16))
    wv = np.ascontiguousarray(np.asarray(Wv, dtype=np.float16))
    bq_ = np.asarray(bq, dtype=np.float32)
    bk_ = np.asarray(bk, dtype=np.float32)
    bv_ = np.asarray(bv, dtype=np.float32)
    assert (
        np.abs(bq_).max() == 0 and np.abs(bk_).max() == 0 and np.abs(bv_).max() == 0
    ), "nonzero qkv bias unsupported in v2 kernel"

    table = np.asarray(rel_pos_bias_table, dtype=np.float32)
    idx = np.asarray(rel_pos_index, dtype=np.int64)
    bias = table[idx.reshape(-1)].reshape(N, N, HEADS)  # [q, k, h]
    biasT = np.exp(bias.transpose(2, 1, 0))  # exp, [h, k, q]
    # masked two-window table: ebm[k 98, oc, hq, q 98]
    ebm = np.zeros((N2, 4, 4, 1, N2), dtype=np.float16)
    for h in range(HEADS):
        oc, hq = divmod(h, 4)
        ebm[0:N, oc, hq, 0, 0:N] = biasT[h]
        ebm[N:N2, oc, hq, 0, N:N2] = biasT[h]

    in_maps = []
    for c in range(NCORES):
        xc = x[c * T : (c + 1) * T]  # [T, DIM]
        xT = np.ascontiguousarray(xc.T.astype(np.float16))  # [DIM, T]
        in_maps.append({"xt": xT, "wq": wq, "wk": wk, "wv": wv, "ebm": ebm})
    return in_maps


def kernel(hidden_states, Wq, bq, Wk, bk, Wv, bv, rel_pos_bias_table, rel_pos_index):
    from concourse.bass_utils import run_bass_kernel_spmd

    in_maps = _host_prep(
        hidden_states, Wq, bq, Wk, bk, Wv, bv, rel_pos_bias_table, rel_pos_index
    )
    if "nc" not in _NC_CACHE:
        _NC_CACHE["nc"] = _build_nc()
    nc = _NC_CACHE["nc"]

    res = run_bass_kernel_spmd(nc, in_maps, core_ids=list(range(NCORES)))
    out = np.empty((B * N, DIM), dtype=np.float32)
    for c in range(NCORES):
        out[c * T : (c + 1) * T] = res.results[c]["out"]
    return out.reshape(B, N, DIM)
